# revision 2
# baseline (speedup 1.0000x reference)
"""MoE layer with MXFP4 expert weights — Trainium2 Bass kernel.

Strategy (expert-parallel, routed, mixed precision):
  - Host: gating (softmax -> top-k -> renorm) with jax on CPU to match the
    reference bitwise; token-expert PAIRS split into a hi set (computed
    with bf16 activations) and a lo set (fp8 activations, DoubleRow
    matmuls at 2x rate). Initially hi = primary (rank-0) pairs and
    lo = secondary; primaries of overfull experts are demoted by lowest
    routing weight to balance hi counts (their combine weight bounds the
    fp8 quantization error contribution).
  - Host: MXFP4 dequant to fp8-e4m3 slab images (near-exact: fp4
    mantissas are exact in e4m3; only deep-subnormal group scales lose
    bits, ~5e-3 end to end).
  - Device (8 cores, SPMD): core e runs expert e's SwiGLU MLP: hi tokens
    via bf16-moving matmuls, lo tokens via fp8 DoubleRow (K=256/instr).
    The lo-path g tile is written directly in e4m3 with a per-expert
    power-of-2 scale (folded into the h3 activation and the output
    activation scale, both free).
  - Host: weighted scatter-add combine in fp32.

Problem shapes (hardcoded): T=1024, H=2048, I=4096, E=8, top_k=2.
"""

import os

import ml_dtypes
import numpy as np

BF16 = ml_dtypes.bfloat16
FP8 = ml_dtypes.float8_e4m3
FP4_VALUES = np.array(
    [0.0, 0.5, 1.0, 1.5, 2.0, 3.0, 4.0, 6.0,
     -0.0, -0.5, -1.0, -1.5, -2.0, -3.0, -4.0, -6.0],
    dtype=np.float32,
)
GROUP_SIZE = 32
T, H, I, E = 1024, 2048, 4096, 8
KH = H // 128   # 16 k-chunks for GEMM1
MI = I // 128   # 32 m-tiles for GEMM1
KI = I // 128   # 32 k-chunks for GEMM2
NH = H // 128   # 16 m-tiles for GEMM2

C_HI_TARGET = 128  # hi-set per-expert cap (demotion threshold)

# bias/scale column layout in bc
BC_B1 = 0
BC_B3 = MI
BC_B2 = 2 * MI
BC_B3S = 2 * MI + NH
BC_SGI = 3 * MI + NH      # 2^-kg column
BC_SGO = 3 * MI + NH + 1  # 2^kg column
BC_COLS = 3 * MI + NH + 2

# test harness hooks
LAST_RESULTS = None


def _split_multiwait_drains(nc):
    """This walrus build only allows 1 sync-wait command per instruction;
    Tile's tail drain carries one wait per active proc. Split the extras
    into single-wait drains placed just before the overfull instruction."""
    import bass_rust

    for f in nc.m.functions:
        blocks = list(f.blocks)
        # snapshot before creating anything: engine.drain() auto-registers
        # new insts at the tail of the current block
        orig = {b.name: list(b.instructions) for b in blocks}
        extras = {}  # (block, inst name) -> [single-wait drains]
        for b in blocks:
            for inst in orig[b.name]:
                si = inst.sync_info
                if si is None or not si.on_wait or len(si.on_wait) <= 1:
                    continue
                # keep only the max wait value per semaphore (sem-ge waits)
                if all(w.wait_mode == "sem-ge-imm" for w in si.on_wait):
                    best = {}
                    for w in si.on_wait:
                        key = w.id
                        if (
                            key not in best
                            or (w.wait_value or 0) > (best[key].wait_value or 0)
                        ):
                            best[key] = w
                    ow = list(best.values())
                else:
                    ow = list(si.on_wait)
                ex = []
                for w in ow[:-1]:
                    d = nc.engines[inst.engine].drain()
                    d.ins.sync_info = bass_rust.SyncInfo(on_wait=[w], on_update=[])
                    ex.append(d.ins)
                si.on_wait = ow[-1:]
                extras[(b.name, inst.name)] = ex
        if not extras:
            continue
        for b in blocks:
            out = []
            for inst in orig[b.name]:
                out.extend(extras.get((b.name, inst.name), ()))
                out.append(inst)
            b.instructions = out


def _routing(hidden_states, gate_weight, top_k):
    """Replicate the reference gating bitwise using jax on CPU."""
    import jax
    import jax.numpy as jnp

    cpu = jax.devices("cpu")[0]
    with jax.default_device(cpu):
        hs = jnp.asarray(hidden_states)
        gw = jnp.asarray(gate_weight)
        logits = hs.astype(jnp.float32) @ gw.T
        probs = jax.nn.softmax(logits, axis=-1)
        w, idx = jax.lax.top_k(probs, top_k)
        w = w / jnp.sum(w, axis=-1, keepdims=True)
    return np.asarray(w), np.asarray(idx)


def _dequant(q, s):
    """q [n, k//2] int32 packed fp4 pairs, s [n, k//32] int32 e8m0.
    Returns exact f32 [n, k]."""
    lo = FP4_VALUES[q & 15]
    hi = FP4_VALUES[(q >> 4) & 15]
    n = q.shape[0]
    vals = np.stack([lo, hi], axis=-1).reshape(n, -1)  # f32 [n, k]
    scale = np.exp2((s - 127).astype(np.float32))
    scale = np.where(s == 0, np.float32(0), scale)
    k = vals.shape[1]
    vals = vals.reshape(n, k // GROUP_SIZE, GROUP_SIZE)
    return (vals * scale[:, :, None]).reshape(n, k)  # f32, exact


def _pack_lhsT(W, n_m, n_k):
    """W [M, K] f32 -> fp8-e4m3 slab images [n_m, 128, n_k*128] where
    slab[m][p, k*128 + f] = W[m*128 + f, k*128 + p] (the SBUF image of
    the pre-transposed stationary operand, contiguous per partition)."""
    Wb = W.astype(FP8)
    arr = Wb.reshape(n_m, 128, n_k, 128).transpose(0, 3, 2, 1)  # [m, p, k, f]
    return np.ascontiguousarray(arr).reshape(n_m, 128, n_k * 128)


def _x_image(X, C, dtype):
    """X [c, H] f32 -> SBUF image [128, KH*C]: [p, k*C + c] = X[c, k*128+p]."""
    cnt = X.shape[0]
    XT = np.zeros((H, C), dtype=dtype)
    XT[:, :cnt] = X.T.astype(dtype)
    return np.ascontiguousarray(
        XT.reshape(KH, 128, C).transpose(1, 0, 2)
    ).reshape(128, KH * C)


_KERNEL_CACHE = {}


def _build_kernel(CP, CS):
    import concourse.bass as bass
    import concourse.mybir as mybir
    import concourse.tile as tile

    bf = mybir.dt.bfloat16
    f8 = mybir.dt.float8e4
    f32 = mybir.dt.float32
    AF = mybir.ActivationFunctionType
    DR = mybir.MatmulPerfMode.DoubleRow

    nc = bass.Bass()
    # SBUF images: [p, k*C + c] = X[c, k*128 + p]
    xtp = nc.dram_tensor("xtp", [128, KH * CP], bf, kind="ExternalInput")
    xts = nc.dram_tensor("xts", [128, KH * CS], f8, kind="ExternalInput")
    # per m-tile: w1 slab then w3 slab concatenated along free dim
    w13s = nc.dram_tensor("w13s", [MI, 128, 2 * KH * 128], f8, kind="ExternalInput")
    w2s = nc.dram_tensor("w2s", [NH, 128, KI * 128], f8, kind="ExternalInput")
    bc = nc.dram_tensor("bc", [128, BC_COLS], f32, kind="ExternalInput")
    ytp = nc.dram_tensor("ytp", [NH, 128, CP], bf, kind="ExternalOutput")
    yts = nc.dram_tensor("yts", [NH, 128, CS], bf, kind="ExternalOutput")

    with tile.TileContext(nc) as tc:
        with (
            tc.tile_pool(name="const", bufs=1) as cpool,
            tc.tile_pool(name="w", bufs=6) as wpool,
            tc.tile_pool(name="act", bufs=3) as spool,
            tc.tile_pool(name="psum", bufs=2, space="PSUM") as ppool,
        ):
            xsp = cpool.tile([128, KH * CP], bf, tag="xsp")
            xss = cpool.tile([128, KH, CS], f8, tag="xss")
            gsp = cpool.tile([128, KI * CP], bf, tag="gsp")
            gss = cpool.tile([128, KI, CS], f8, tag="gss")
            bt = cpool.tile([128, BC_COLS], f32, tag="bt")

            # PE pre-warm while the head DMAs land
            warm = cpool.tile([128, 128], bf, tag="warm")
            nc.gpsimd.memset(warm[:], 0.0)
            wp = ppool.tile([128, 128], f32, tag="wp")
            N_WARM = 40
            for i in range(N_WARM):
                nc.tensor.matmul(
                    wp[:], warm[:], warm[:], start=(i == 0),
                    stop=(i == N_WARM - 1),
                )

            # head: activations + biases on the SP ring, first weight slab
            # in pieces on the ACT ring in parallel
            XP = 2
            xpc = KH // XP
            nc.sync.dma_start(xsp[:, : xpc * CP], xtp[:, : xpc * CP])
            w13_0 = wpool.tile([128, 2 * KH, 128], f8, tag="w13")
            WPC = 2 * KH // 4
            for i in range(4):
                nc.scalar.dma_start(
                    w13_0[:, i * WPC:(i + 1) * WPC, :],
                    w13s[0][:, i * WPC * 128:(i + 1) * WPC * 128],
                )
            for i in range(1, XP):
                nc.sync.dma_start(
                    xsp[:, i * xpc * CP:(i + 1) * xpc * CP],
                    xtp[:, i * xpc * CP:(i + 1) * xpc * CP],
                )
            nc.sync.dma_start(xss[:], xts[:])
            nc.sync.dma_start(bt[:], bc[:])

            # GEMM1 + SwiGLU
            for m in range(MI):
                if m == 0:
                    w13t = w13_0
                else:
                    w13t = wpool.tile([128, 2 * KH, 128], f8, tag="w13")
                    eng = nc.sync if m % 2 == 0 else nc.scalar
                    eng.dma_start(w13t[:, :KH, :], w13s[m][:, : KH * 128])
                    eng.dma_start(w13t[:, KH:, :], w13s[m][:, KH * 128:])

                h1p = ppool.tile([128, CP], f32, tag="h1p")
                for k in range(KH):
                    nc.tensor.matmul(
                        h1p[:], w13t[:, k, :],
                        xsp[:, k * CP:(k + 1) * CP],
                        start=(k == 0), stop=(k == KH - 1),
                    )
                h3p = ppool.tile([128, CP], f32, tag="h3p")
                for k in range(KH):
                    nc.tensor.matmul(
                        h3p[:], w13t[:, KH + k, :],
                        xsp[:, k * CP:(k + 1) * CP],
                        start=(k == 0), stop=(k == KH - 1),
                    )
                h1s = ppool.tile([128, CS], f32, tag="h1s")
                for j in range(KH // 2):
                    nc.tensor.matmul(
                        h1s[:], w13t[:, 2 * j:2 * j + 2, :],
                        xss[:, 2 * j:2 * j + 2, :],
                        start=(j == 0), stop=(j == KH // 2 - 1),
                        perf_mode=DR,
                    )
                h3s = ppool.tile([128, CS], f32, tag="h3s")
                for j in range(KH // 2):
                    nc.tensor.matmul(
                        h3s[:], w13t[:, KH + 2 * j:KH + 2 * j + 2, :],
                        xss[:, 2 * j:2 * j + 2, :],
                        start=(j == 0), stop=(j == KH // 2 - 1),
                        perf_mode=DR,
                    )

                t1 = spool.tile([128, CP], bf, tag="t1")
                nc.scalar.activation(
                    t1[:], h1p[:], AF.Silu, bias=bt[:, BC_B1 + m:BC_B1 + m + 1]
                )
                t3 = spool.tile([128, CP], bf, tag="t3")
                nc.scalar.activation(
                    t3[:], h3p[:], AF.Identity, bias=bt[:, BC_B3 + m:BC_B3 + m + 1]
                )
                nc.vector.tensor_mul(
                    gsp[:, m * CP:(m + 1) * CP], t1[:], t3[:]
                )

                t1s = spool.tile([128, CS], bf, tag="t1s")
                nc.scalar.activation(
                    t1s[:], h1s[:], AF.Silu, bias=bt[:, BC_B1 + m:BC_B1 + m + 1]
                )
                t3s = spool.tile([128, CS], bf, tag="t3s")
                nc.scalar.activation(
                    t3s[:], h3s[:], AF.Identity,
                    bias=bt[:, BC_B3S + m:BC_B3S + m + 1],
                    scale=bt[:, BC_SGI:BC_SGI + 1],
                )
                nc.vector.tensor_mul(gss[:, m, :], t1s[:], t3s[:])

            # GEMM2 + bias
            for n in range(NH):
                w2t = wpool.tile([128, KI, 128], f8, tag="w2")
                eng = nc.sync if n % 2 == 0 else nc.scalar
                eng.dma_start(w2t[:], w2s[n])

                op = ppool.tile([128, CP], f32, tag="h1p")
                for k in range(KI):
                    nc.tensor.matmul(
                        op[:], w2t[:, k, :],
                        gsp[:, k * CP:(k + 1) * CP],
                        start=(k == 0), stop=(k == KI - 1),
                    )
                ops = ppool.tile([128, CS], f32, tag="h1s")
                for j in range(KI // 2):
                    nc.tensor.matmul(
                        ops[:], w2t[:, 2 * j:2 * j + 2, :],
                        gss[:, 2 * j:2 * j + 2, :],
                        start=(j == 0), stop=(j == KI // 2 - 1),
                        perf_mode=DR,
                    )

                yo = spool.tile([128, CP], bf, tag="y")
                nc.scalar.activation(
                    yo[:], op[:], AF.Identity, bias=bt[:, BC_B2 + n:BC_B2 + n + 1]
                )
                nc.sync.dma_start(ytp[n], yo[:])
                yos = spool.tile([128, CS], bf, tag="ys")
                nc.scalar.activation(
                    yos[:], ops[:], AF.Identity,
                    bias=bt[:, BC_B2 + n:BC_B2 + n + 1],
                    scale=bt[:, BC_SGO:BC_SGO + 1],
                )
                nc.sync.dma_start(yts[n], yos[:])

    _split_multiwait_drains(nc)
    return nc


def kernel(hidden_states, gate_weight, w1_weight, w3_weight, w2_weight,
           w13_scale, w2_scale, w13_bias, w2_bias, top_k):
    global LAST_RESULTS
    from concourse.bass_utils import run_bass_kernel_spmd

    hs = np.asarray(hidden_states)
    gw = np.asarray(gate_weight, dtype=np.float32)
    w1q = np.asarray(w1_weight)
    w3q = np.asarray(w3_weight)
    w2q = np.asarray(w2_weight)
    s13 = np.asarray(w13_scale)
    s2 = np.asarray(w2_scale)
    b13 = np.asarray(w13_bias)
    b2 = np.asarray(w2_bias)
    K = int(top_k)

    # ---- routing on host (bitwise-matches reference) ----
    w, idx = _routing(hs, gw, K)

    # hi/lo pair assignment with demotion balancing
    hi = [[] for _ in range(E)]
    lo = [[] for _ in range(E)]
    for t in range(T):
        hi[idx[t, 0]].append((t, float(w[t, 0])))
        for r in range(1, K):
            lo[idx[t, r]].append((t, float(w[t, r])))
    for e in range(E):
        if len(hi[e]) > C_HI_TARGET:
            hi[e].sort(key=lambda tw: tw[1])
            n_dem = len(hi[e]) - C_HI_TARGET
            lo[e].extend(hi[e][:n_dem])
            hi[e] = hi[e][n_dem:]
    CP = max(32, -(-max(len(x) for x in hi) // 8) * 8)
    CS = max(32, -(-max(len(x) for x in lo) // 8) * 8)

    hsf = hs.astype(np.float32)
    xn_max = float(np.linalg.norm(hsf, axis=1).max())

    # ---- per-expert input packing ----
    in_maps = []
    for e in range(E):
        W1 = _dequant(w1q[e], s13[e, :I])       # f32 [I, H]
        W3 = _dequant(w3q[e], s13[e, I:])       # f32 [I, H]
        W2 = _dequant(w2q[e], s2[e])            # f32 [H, I]
        b1 = b13[e, :I].astype(np.float32)
        b3 = b13[e, I:].astype(np.float32)
        bb2 = b2[e].astype(np.float32)

        # power-of-2 bound on |g| for the lo path (Cauchy-Schwarz)
        W1q8 = W1.astype(FP8).astype(np.float32)
        W3q8 = W3.astype(FP8).astype(np.float32)
        h1b = np.linalg.norm(W1q8, axis=1) * xn_max + np.abs(b1)
        h3b = np.linalg.norm(W3q8, axis=1) * xn_max + np.abs(b3)
        Bg = float((h1b * h3b).max())
        kg = max(0.0, float(np.ceil(np.log2(max(Bg, 1e-30) / 224.0))))
        sgi = np.float32(2.0 ** -kg)
        sgo = np.float32(2.0 ** kg)

        te_hi = np.array([t for t, _ in hi[e]], dtype=np.int64)
        te_lo = np.array([t for t, _ in lo[e]], dtype=np.int64)
        Xhi = hsf[te_hi] if len(te_hi) else np.zeros((0, H), np.float32)
        Xlo = hsf[te_lo] if len(te_lo) else np.zeros((0, H), np.float32)

        bcols = np.zeros((128, BC_COLS), dtype=np.float32)
        bcols[:, BC_B1:BC_B1 + MI] = b1.reshape(MI, 128).T
        bcols[:, BC_B3:BC_B3 + MI] = b3.reshape(MI, 128).T
        bcols[:, BC_B2:BC_B2 + NH] = bb2.reshape(NH, 128).T
        bcols[:, BC_B3S:BC_B3S + MI] = (b3 * sgi).reshape(MI, 128).T
        bcols[:, BC_SGI] = sgi
        bcols[:, BC_SGO] = sgo

        in_maps.append({
            "xtp": _x_image(Xhi, CP, BF16),
            "xts": _x_image(Xlo, CS, FP8),
            "w13s": np.ascontiguousarray(np.concatenate(
                [_pack_lhsT(W1, MI, KH), _pack_lhsT(W3, MI, KH)], axis=2
            )),
            "w2s": _pack_lhsT(W2, NH, KI),
            "bc": np.ascontiguousarray(bcols),
        })

    # ---- build + run on 8 cores ----
    key = (CP, CS)
    if key not in _KERNEL_CACHE:
        _KERNEL_CACHE[key] = _build_kernel(CP, CS)
    nc = _KERNEL_CACHE[key]

    trace = os.environ.get("MOE_TRACE") == "1"
    kw = {}
    if trace and os.environ.get("MOE_TRACE_ALL") == "1":
        kw["trace_cores"] = list(range(E))
    res = run_bass_kernel_spmd(
        nc, in_maps, core_ids=list(range(E)), trace=trace, **kw
    )
    LAST_RESULTS = res

    # ---- weighted combine on host (fp32, like the reference) ----
    final = np.zeros((T, H), dtype=np.float32)
    for e in range(E):
        for pairs, out_name, C in ((hi[e], "ytp", CP), (lo[e], "yts", CS)):
            if not pairs:
                continue
            te = np.array([t for t, _ in pairs], dtype=np.int64)
            we = np.array([ww for _, ww in pairs], dtype=np.float32)
            Y = res.results[e][out_name].reshape(NH, 128, C)
            Y = Y.transpose(0, 1, 2).reshape(H, C)[:, :len(te)]
            final[te] += we[:, None] * Y.T.astype(np.float32)
    return final.astype(BF16)


# revision 6
# speedup vs baseline: 1.2712x; 1.2712x over previous
"""MoE layer with MXFP4 expert weights — Trainium2 Bass kernel.

Strategy (expert-parallel, routed, mixed precision):
  - Host: gating (softmax -> top-k -> renorm) with jax on CPU to match the
    reference bitwise; token-expert PAIRS split into a hi set (computed
    with bf16 activations) and a lo set (fp8 activations, DoubleRow
    matmuls at 2x rate). Initially hi = primary (rank-0) pairs and
    lo = secondary; primaries of overfull experts are demoted by lowest
    routing weight to balance hi counts (their combine weight bounds the
    fp8 quantization error contribution).
  - Host: MXFP4 dequant to fp8-e4m3 slab images (near-exact: fp4
    mantissas are exact in e4m3; only deep-subnormal group scales lose
    bits, ~5e-3 end to end).
  - Device (8 cores, SPMD): core e runs expert e's SwiGLU MLP: hi tokens
    via bf16-moving matmuls, lo tokens via fp8 DoubleRow (K=256/instr).
    The lo-path g tile is written directly in e4m3 with a per-expert
    power-of-2 scale (folded into the h3 activation and the output
    activation scale, both free).
  - Host: weighted scatter-add combine in fp32.

Problem shapes (hardcoded): T=1024, H=2048, I=4096, E=8, top_k=2.
"""

import os

import ml_dtypes
import numpy as np

BF16 = ml_dtypes.bfloat16
FP8 = ml_dtypes.float8_e4m3
FP4_VALUES = np.array(
    [0.0, 0.5, 1.0, 1.5, 2.0, 3.0, 4.0, 6.0,
     -0.0, -0.5, -1.0, -1.5, -2.0, -3.0, -4.0, -6.0],
    dtype=np.float32,
)
GROUP_SIZE = 32
T, H, I, E = 1024, 2048, 4096, 8
KH = H // 128   # 16 k-chunks for GEMM1
MI = I // 128   # 32 m-tiles for GEMM1
KI = I // 128   # 32 k-chunks for GEMM2
NH = H // 128   # 16 m-tiles for GEMM2

C_HI_TARGET = 128  # hi-set per-expert cap (demotion threshold)

# bias/scale column layout in bc
BC_B1 = 0
BC_B3 = MI
BC_B2 = 2 * MI
BC_B3S = 2 * MI + NH
BC_SGI = 3 * MI + NH      # 2^-kg column
BC_SGO = 3 * MI + NH + 1  # 2^kg column
BC_COLS = 3 * MI + NH + 2

# test harness hooks
LAST_RESULTS = None


def _split_multiwait_drains(nc):
    """This walrus build only allows 1 sync-wait command per instruction;
    Tile's tail drain carries one wait per active proc. Split the extras
    into single-wait drains placed just before the overfull instruction."""
    import bass_rust

    for f in nc.m.functions:
        blocks = list(f.blocks)
        # snapshot before creating anything: engine.drain() auto-registers
        # new insts at the tail of the current block
        orig = {b.name: list(b.instructions) for b in blocks}
        extras = {}  # (block, inst name) -> [single-wait drains]
        for b in blocks:
            for inst in orig[b.name]:
                si = inst.sync_info
                if si is None or not si.on_wait or len(si.on_wait) <= 1:
                    continue
                # keep only the max wait value per semaphore (sem-ge waits)
                if all(w.wait_mode == "sem-ge-imm" for w in si.on_wait):
                    best = {}
                    for w in si.on_wait:
                        key = w.id
                        if (
                            key not in best
                            or (w.wait_value or 0) > (best[key].wait_value or 0)
                        ):
                            best[key] = w
                    ow = list(best.values())
                else:
                    ow = list(si.on_wait)
                ex = []
                for w in ow[:-1]:
                    d = nc.engines[inst.engine].drain()
                    d.ins.sync_info = bass_rust.SyncInfo(on_wait=[w], on_update=[])
                    ex.append(d.ins)
                si.on_wait = ow[-1:]
                extras[(b.name, inst.name)] = ex
        if not extras:
            continue
        for b in blocks:
            out = []
            for inst in orig[b.name]:
                out.extend(extras.get((b.name, inst.name), ()))
                out.append(inst)
            b.instructions = out


def _routing(hidden_states, gate_weight, top_k):
    """Replicate the reference gating bitwise using jax on CPU."""
    import jax
    import jax.numpy as jnp

    cpu = jax.devices("cpu")[0]
    with jax.default_device(cpu):
        hs = jnp.asarray(hidden_states)
        gw = jnp.asarray(gate_weight)
        logits = hs.astype(jnp.float32) @ gw.T
        probs = jax.nn.softmax(logits, axis=-1)
        w, idx = jax.lax.top_k(probs, top_k)
        w = w / jnp.sum(w, axis=-1, keepdims=True)
    return np.asarray(w), np.asarray(idx)


def _dequant(q, s):
    """q [n, k//2] int32 packed fp4 pairs, s [n, k//32] int32 e8m0.
    Returns exact f32 [n, k]."""
    lo = FP4_VALUES[q & 15]
    hi = FP4_VALUES[(q >> 4) & 15]
    n = q.shape[0]
    vals = np.stack([lo, hi], axis=-1).reshape(n, -1)  # f32 [n, k]
    scale = np.exp2((s - 127).astype(np.float32))
    scale = np.where(s == 0, np.float32(0), scale)
    k = vals.shape[1]
    vals = vals.reshape(n, k // GROUP_SIZE, GROUP_SIZE)
    return (vals * scale[:, :, None]).reshape(n, k)  # f32, exact


def _pack_lhsT(W, n_m, n_k):
    """W [M, K] f32 -> fp8-e4m3 slab images [n_m, 128, n_k*128] where
    slab[m][p, k*128 + f] = W[m*128 + f, k*128 + p] (the SBUF image of
    the pre-transposed stationary operand, contiguous per partition)."""
    Wb = W.astype(FP8)
    arr = Wb.reshape(n_m, 128, n_k, 128).transpose(0, 3, 2, 1)  # [m, p, k, f]
    return np.ascontiguousarray(arr).reshape(n_m, 128, n_k * 128)


def _x_image(X, C, dtype):
    """X [c, H] f32 -> SBUF image [128, KH*C]: [p, k*C + c] = X[c, k*128+p]."""
    cnt = X.shape[0]
    XT = np.zeros((H, C), dtype=dtype)
    XT[:, :cnt] = X.T.astype(dtype)
    return np.ascontiguousarray(
        XT.reshape(KH, 128, C).transpose(1, 0, 2)
    ).reshape(128, KH * C)


_KERNEL_CACHE = {}


def _build_kernel(CP, CS):
    import concourse.bass as bass
    import concourse.mybir as mybir
    import concourse.tile as tile

    bf = mybir.dt.bfloat16
    f8 = mybir.dt.float8e4
    f32 = mybir.dt.float32
    AF = mybir.ActivationFunctionType
    DR = mybir.MatmulPerfMode.DoubleRow

    nc = bass.Bass()
    # SBUF images: [p, k*C + c] = X[c, k*128 + p]
    xtp = nc.dram_tensor("xtp", [128, KH * CP], bf, kind="ExternalInput")
    xts = nc.dram_tensor("xts", [128, KH * CS], f8, kind="ExternalInput")
    # per m-tile: w1 slab then w3 slab concatenated along free dim
    w13s = nc.dram_tensor("w13s", [MI, 128, 2 * KH * 128], f8, kind="ExternalInput")
    w2s = nc.dram_tensor("w2s", [NH, 128, KI * 128], f8, kind="ExternalInput")
    bc = nc.dram_tensor("bc", [128, BC_COLS], f32, kind="ExternalInput")
    ytp = nc.dram_tensor("ytp", [NH, 128, CP], bf, kind="ExternalOutput")
    yts = nc.dram_tensor("yts", [NH, 128, CS], bf, kind="ExternalOutput")

    with tile.TileContext(nc) as tc:
        with (
            tc.tile_pool(name="const", bufs=1) as cpool,
            tc.tile_pool(name="w", bufs=6) as wpool,
            tc.tile_pool(name="act", bufs=3) as spool,
            tc.tile_pool(name="psum", bufs=2, space="PSUM") as ppool,
        ):
            xsp = cpool.tile([128, KH * CP], bf, tag="xsp")
            xss = cpool.tile([128, KH, CS], f8, tag="xss")
            gsp = cpool.tile([128, KI * CP], bf, tag="gsp")
            gss = cpool.tile([128, KI, CS], f8, tag="gss")
            bt = cpool.tile([128, BC_COLS], f32, tag="bt")

            # PE pre-warm while the head DMAs land (PSUM has 8 banks and
            # the 4 h-tags x 2 bufs use all of them, so warm into h1p)
            warm = cpool.tile([128, 128], bf, tag="warm")
            warm_mv = cpool.tile([128, CP], bf, tag="warm_mv")
            nc.gpsimd.memset(warm[:], 0.0)
            nc.gpsimd.memset(warm_mv[:], 0.0)
            wp = ppool.tile([128, CP], f32, tag="h1p")
            N_WARM = 64
            for i in range(N_WARM):
                nc.tensor.matmul(
                    wp[:], warm[:], warm_mv[:], start=(i == 0),
                    stop=(i == N_WARM - 1),
                )

            # head: activations + biases on the SP ring, first weight slab
            # in pieces on the ACT ring in parallel
            XP = 2
            xpc = KH // XP
            nc.sync.dma_start(xsp[:, : xpc * CP], xtp[:, : xpc * CP])
            w13_0 = wpool.tile([128, 2 * KH, 128], f8, tag="w13")
            WPC = 2 * KH // 4
            for i in range(4):
                nc.scalar.dma_start(
                    w13_0[:, i * WPC:(i + 1) * WPC, :],
                    w13s[0][:, i * WPC * 128:(i + 1) * WPC * 128],
                )
            for i in range(1, XP):
                nc.sync.dma_start(
                    xsp[:, i * xpc * CP:(i + 1) * xpc * CP],
                    xtp[:, i * xpc * CP:(i + 1) * xpc * CP],
                )
            nc.sync.dma_start(xss[:], xts[:])
            nc.sync.dma_start(bt[:], bc[:])

            # GEMM1 + SwiGLU
            for m in range(MI):
                if m == 0:
                    w13t = w13_0
                else:
                    w13t = wpool.tile([128, 2 * KH, 128], f8, tag="w13")
                    eng = nc.sync if m % 2 == 0 else nc.scalar
                    eng.dma_start(w13t[:, :KH, :], w13s[m][:, : KH * 128])
                    eng.dma_start(w13t[:, KH:, :], w13s[m][:, KH * 128:])

                h1p = ppool.tile([128, CP], f32, tag="h1p")
                for k in range(KH):
                    nc.tensor.matmul(
                        h1p[:], w13t[:, k, :],
                        xsp[:, k * CP:(k + 1) * CP],
                        start=(k == 0), stop=(k == KH - 1),
                    )
                h3p = ppool.tile([128, CP], f32, tag="h3p")
                for k in range(KH):
                    nc.tensor.matmul(
                        h3p[:], w13t[:, KH + k, :],
                        xsp[:, k * CP:(k + 1) * CP],
                        start=(k == 0), stop=(k == KH - 1),
                    )
                h1s = ppool.tile([128, CS], f32, tag="h1s")
                for j in range(KH // 2):
                    nc.tensor.matmul(
                        h1s[:], w13t[:, 2 * j:2 * j + 2, :],
                        xss[:, 2 * j:2 * j + 2, :],
                        start=(j == 0), stop=(j == KH // 2 - 1),
                        perf_mode=DR,
                    )
                h3s = ppool.tile([128, CS], f32, tag="h3s")
                for j in range(KH // 2):
                    nc.tensor.matmul(
                        h3s[:], w13t[:, KH + 2 * j:KH + 2 * j + 2, :],
                        xss[:, 2 * j:2 * j + 2, :],
                        start=(j == 0), stop=(j == KH // 2 - 1),
                        perf_mode=DR,
                    )

                t1 = spool.tile([128, CP], bf, tag="t1")
                nc.scalar.activation(
                    t1[:], h1p[:], AF.Silu, bias=bt[:, BC_B1 + m:BC_B1 + m + 1]
                )
                t3 = spool.tile([128, CP], bf, tag="t3")
                nc.scalar.activation(
                    t3[:], h3p[:], AF.Identity, bias=bt[:, BC_B3 + m:BC_B3 + m + 1]
                )
                nc.vector.tensor_mul(
                    gsp[:, m * CP:(m + 1) * CP], t1[:], t3[:]
                )

                t1s = spool.tile([128, CS], bf, tag="t1s")
                nc.scalar.activation(
                    t1s[:], h1s[:], AF.Silu, bias=bt[:, BC_B1 + m:BC_B1 + m + 1]
                )
                t3s = spool.tile([128, CS], bf, tag="t3s")
                nc.scalar.activation(
                    t3s[:], h3s[:], AF.Identity,
                    bias=bt[:, BC_B3S + m:BC_B3S + m + 1],
                    scale=bt[:, BC_SGI:BC_SGI + 1],
                )
                gtmp = spool.tile([128, CS], bf, tag="gtmp")
                nc.vector.tensor_mul(gtmp[:], t1s[:], t3s[:])
                # clamp to the e4m3 safe range and convert to fp8
                nc.vector.tensor_scalar(
                    gss[:, m, :], gtmp[:], 224.0, -224.0,
                    op0=mybir.AluOpType.min, op1=mybir.AluOpType.max,
                )

            # GEMM2 + bias
            for n in range(NH):
                w2t = wpool.tile([128, KI, 128], f8, tag="w2")
                eng = nc.sync if n % 2 == 0 else nc.scalar
                eng.dma_start(w2t[:], w2s[n])

                op = ppool.tile([128, CP], f32, tag="h1p")
                for k in range(KI):
                    nc.tensor.matmul(
                        op[:], w2t[:, k, :],
                        gsp[:, k * CP:(k + 1) * CP],
                        start=(k == 0), stop=(k == KI - 1),
                    )
                ops = ppool.tile([128, CS], f32, tag="h1s")
                for j in range(KI // 2):
                    nc.tensor.matmul(
                        ops[:], w2t[:, 2 * j:2 * j + 2, :],
                        gss[:, 2 * j:2 * j + 2, :],
                        start=(j == 0), stop=(j == KI // 2 - 1),
                        perf_mode=DR,
                    )

                yo = spool.tile([128, CP], bf, tag="y")
                nc.scalar.activation(
                    yo[:], op[:], AF.Identity, bias=bt[:, BC_B2 + n:BC_B2 + n + 1]
                )
                nc.sync.dma_start(ytp[n], yo[:])
                yos = spool.tile([128, CS], bf, tag="ys")
                nc.scalar.activation(
                    yos[:], ops[:], AF.Identity,
                    bias=bt[:, BC_B2 + n:BC_B2 + n + 1],
                    scale=bt[:, BC_SGO:BC_SGO + 1],
                )
                nc.sync.dma_start(yts[n], yos[:])

    _split_multiwait_drains(nc)
    return nc


def kernel(hidden_states, gate_weight, w1_weight, w3_weight, w2_weight,
           w13_scale, w2_scale, w13_bias, w2_bias, top_k):
    global LAST_RESULTS
    from concourse.bass_utils import run_bass_kernel_spmd

    hs = np.asarray(hidden_states)
    gw = np.asarray(gate_weight, dtype=np.float32)
    w1q = np.asarray(w1_weight)
    w3q = np.asarray(w3_weight)
    w2q = np.asarray(w2_weight)
    s13 = np.asarray(w13_scale)
    s2 = np.asarray(w2_scale)
    b13 = np.asarray(w13_bias)
    b2 = np.asarray(w2_bias)
    K = int(top_k)

    # ---- routing on host (bitwise-matches reference) ----
    w, idx = _routing(hs, gw, K)

    # hi/lo pair assignment with demotion balancing
    hi = [[] for _ in range(E)]
    lo = [[] for _ in range(E)]
    for t in range(T):
        hi[idx[t, 0]].append((t, float(w[t, 0])))
        for r in range(1, K):
            lo[idx[t, r]].append((t, float(w[t, r])))
    for e in range(E):
        if len(hi[e]) > C_HI_TARGET:
            hi[e].sort(key=lambda tw: tw[1])
            n_dem = len(hi[e]) - C_HI_TARGET
            lo[e].extend(hi[e][:n_dem])
            hi[e] = hi[e][n_dem:]
    CP = max(32, -(-max(len(x) for x in hi) // 8) * 8)
    CS = max(32, -(-max(len(x) for x in lo) // 8) * 8)

    hsf = hs.astype(np.float32)

    # ---- per-expert input packing ----
    in_maps = []
    for e in range(E):
        W1 = _dequant(w1q[e], s13[e, :I])       # f32 [I, H]
        W3 = _dequant(w3q[e], s13[e, I:])       # f32 [I, H]
        W2 = _dequant(w2q[e], s2[e])            # f32 [H, I]
        b1 = b13[e, :I].astype(np.float32)
        b3 = b13[e, I:].astype(np.float32)
        bb2 = b2[e].astype(np.float32)

        te_hi = np.array([t for t, _ in hi[e]], dtype=np.int64)
        te_lo = np.array([t for t, _ in lo[e]], dtype=np.int64)
        Xhi = hsf[te_hi] if len(te_hi) else np.zeros((0, H), np.float32)
        Xlo = hsf[te_lo] if len(te_lo) else np.zeros((0, H), np.float32)

        # power-of-2 scale for the lo-path g: sampled max (largest-norm
        # tokens + random) plus margin octaves; on-device clamp covers the
        # residual tail
        W1q8 = W1.astype(FP8).astype(np.float32)
        W3q8 = W3.astype(FP8).astype(np.float32)
        if len(te_lo):
            xn = np.linalg.norm(Xlo, axis=1)
            samp = np.argsort(-xn)[:16]
            rng = np.random.default_rng(e)
            extra = rng.choice(len(te_lo), min(16, len(te_lo)), replace=False)
            sel = np.unique(np.concatenate([samp, extra]))
            Xs = Xlo[sel].astype(FP8).astype(np.float32)
            h1 = Xs @ W1q8.T + b1
            h3 = Xs @ W3q8.T + b3
            gmax = float(np.abs(
                (h1 / (1.0 + np.exp(-np.clip(h1, -80, 80)))) * h3
            ).max())
        else:
            gmax = 1.0
        kg = max(0.0, float(np.ceil(np.log2(max(gmax, 1e-30) / 224.0)))) + 3.0
        sgi = np.float32(2.0 ** -kg)
        sgo = np.float32(2.0 ** kg)

        bcols = np.zeros((128, BC_COLS), dtype=np.float32)
        bcols[:, BC_B1:BC_B1 + MI] = b1.reshape(MI, 128).T
        bcols[:, BC_B3:BC_B3 + MI] = b3.reshape(MI, 128).T
        bcols[:, BC_B2:BC_B2 + NH] = bb2.reshape(NH, 128).T
        bcols[:, BC_B3S:BC_B3S + MI] = (b3 * sgi).reshape(MI, 128).T
        bcols[:, BC_SGI] = sgi
        bcols[:, BC_SGO] = sgo

        in_maps.append({
            "xtp": _x_image(Xhi, CP, BF16),
            "xts": _x_image(Xlo, CS, FP8),
            "w13s": np.ascontiguousarray(np.concatenate(
                [_pack_lhsT(W1, MI, KH), _pack_lhsT(W3, MI, KH)], axis=2
            )),
            "w2s": _pack_lhsT(W2, NH, KI),
            "bc": np.ascontiguousarray(bcols),
        })

    # ---- build + run on 8 cores ----
    key = (CP, CS)
    if key not in _KERNEL_CACHE:
        _KERNEL_CACHE[key] = _build_kernel(CP, CS)
    nc = _KERNEL_CACHE[key]

    trace = os.environ.get("MOE_TRACE") == "1"
    kw = {}
    if trace and os.environ.get("MOE_TRACE_ALL") == "1":
        kw["trace_cores"] = list(range(E))
    res = run_bass_kernel_spmd(
        nc, in_maps, core_ids=list(range(E)), trace=trace, **kw
    )
    LAST_RESULTS = res

    # ---- weighted combine on host (fp32, like the reference) ----
    final = np.zeros((T, H), dtype=np.float32)
    for e in range(E):
        for pairs, out_name, C in ((hi[e], "ytp", CP), (lo[e], "yts", CS)):
            if not pairs:
                continue
            te = np.array([t for t, _ in pairs], dtype=np.int64)
            we = np.array([ww for _, ww in pairs], dtype=np.float32)
            Y = res.results[e][out_name].reshape(NH, 128, C)
            Y = Y.transpose(0, 1, 2).reshape(H, C)[:, :len(te)]
            final[te] += we[:, None] * Y.T.astype(np.float32)
    return final.astype(BF16)
